# revision 69
# baseline (speedup 1.0000x reference)
"""Causal cross-attention Trainium2 kernel (8-core SPMD).

Problem: B=2, T=T_ctx=2048, C=1024, H=16 heads, D=64.
  q = x@Wq + bq;  k,v = context@Wkv + bkv
  att = softmax(causal_mask(q k^T / sqrt(D)));  out = (att v) @ Wp + bp

Sharding (data parallel on B x tensor parallel on heads):
  core c: batch b = c // 4, heads [4*(c%4) .. 4*(c%4)+3]
  Each core computes q/k/v projections for its 256 head-dim columns,
  attention for its 4 heads, and a partial out-projection (its rows of
  Wp). Host transposes x/context per batch and casts activations +
  q/k/v weights to bf16 (pure layout/dtype prep), sums the 4 partial
  outputs per batch, and adds bp.

Per-core dataflow:
  qT/kT in [dc, t] layout (head dims on partitions, 2 m-tiles of 128),
  v_aug in [s, 65*h] layout (64 v cols + ones col per head -> softmax
  denominators fall out of the AV matmul). Attention runs in THREE
  t-phases (t<1024, [1024,1536), [1536,2048)) so each phase's exp work
  (ACT engine, the attention-region bottleneck) overlaps PE-heavy
  independent work: projection chunks in phase A, out-projection tiles
  of earlier phases in B/C. Per phase+head, scores^T [s, t] causal
  spans are matmul'd into [128,1024] PSUM tiles (two 512-col chunks
  per tile when possible -> one ACT exp instruction per tile), exp'd
  into per-phase ping-pong expT buffers (heads alternate, so head h+1's
  exps overlap head h's AV), AV accumulates [65, 512] per q-block in
  PSUM (row 64 = denominator), normalized via DVE reciprocal + a K=1
  PE broadcast matmul (normalization deferred one q-block so PE never
  waits on the reciprocal round-trip).
"""
import sys

sys.path.insert(0, '/opt/trn_rl_repo')

import ml_dtypes
import numpy as np

import concourse.bass as bass
import concourse.mybir as mybir
from concourse.tile import TileContext

F32 = mybir.dt.float32
F32R = mybir.dt.float32r
BF16 = mybir.dt.bfloat16
EXP = mybir.ActivationFunctionType.Exp
COPY = mybir.ActivationFunctionType.Copy

B, T, C, H, D = 2, 2048, 1024, 16, 64
HC = 4            # heads per core
DC = HC * D       # head-dim columns per core (256)
VC = HC * 65      # v_aug columns (per head: 64 v cols + ones col)
NT = T // 128     # 16 s/t tiles
KO = C // 128     # 8 contraction subtiles

_cached = {}


def split_sync_waits(nc, maxw=1):
    """This walrus build rejects instructions with >1 sync-wait; move the
    excess onto dedicated NOPs inserted just before, on the same engine."""
    n = 0
    for fn in nc.m.functions:
        for bb in fn.blocks:
            insts = bb.instructions
            i = 0
            while i < len(insts):
                inst = insts[i]
                si = getattr(inst, 'sync_info', None)
                if si is not None and si.on_wait and len(si.on_wait) > maxw:
                    waits = list(si.on_wait)
                    extra = waits[:-maxw]
                    while len(si.on_wait) > maxw:
                        si.on_wait.pop(0)
                    nops = []
                    for w in extra:
                        nop = mybir.InstNoOp(
                            name=f"I-{nc.next_id()}",
                            engine=inst.engine,
                            bass_nofuse=True,
                            sync_info=mybir.SyncInfo(on_wait=[w], on_update=[]),
                        )
                        nc.register_instruction(nop)
                        nops.append(nop)
                    insts[i:i] = nops
                    i += len(nops)
                    n += 1
                i += 1
    return n


# Three t-phases. Each phase stores, per s-tile j, the causal span
# [max(128j, TLO), THI) contiguously in its expT buffer. Score chunks are
# <=512 cols (PSUM bank); two consecutive chunks share a [128,1024] PSUM
# tile (first chunk must be 512 wide so the second starts bank-aligned
# and the pair stays contiguous for a single exp).
PH = []
for _TLO, _THI, _qbs in ((0, 1024, (0, 1)), (1024, 1536, (2,)),
                         (1536, 2048, (3,))):
    _js = [j for j in range(NT) if _THI - max(128 * j, _TLO) > 0]
    _offs = {}
    _off = 0
    _chunks = []           # (j, c0_in_span, width, expT_off)
    for _j in _js:
        _offs[_j] = _off
        _span = _THI - max(128 * _j, _TLO)
        _c0 = 0
        while _c0 < _span:
            _w = min(512, _span - _c0)
            _chunks.append((_j, _c0, _w, _off + _c0))
            _c0 += _w
        _off += _span
    _groups = []
    _i = 0
    while _i < len(_chunks):
        if _i + 1 < len(_chunks):
            _groups.append([_chunks[_i], _chunks[_i + 1]])
            _i += 2
        else:
            _groups.append([_chunks[_i]])
            _i += 1
    PH.append(dict(TLO=_TLO, THI=_THI, qbs=_qbs, js=_js, OFF=_offs,
                   COLS=_off, groups=_groups))


def build_program(has_bias=True):
    nc = bass.Bass()

    xT_d = nc.dram_tensor("xT", [C, T], BF16, kind="ExternalInput")
    cT_d = nc.dram_tensor("cT", [C, T], BF16, kind="ExternalInput")
    wq_d = nc.dram_tensor("wq", [C, DC], BF16, kind="ExternalInput")
    wk_d = nc.dram_tensor("wk", [C, DC], BF16, kind="ExternalInput")
    wv_d = nc.dram_tensor("wv", [C, VC], BF16, kind="ExternalInput")
    wp_d = nc.dram_tensor("wp", [DC, C], BF16, kind="ExternalInput")
    msk_d = nc.dram_tensor("msk", [128, 128], F32R, kind="ExternalInput")
    idn_d = nc.dram_tensor("idn", [128, 128], BF16, kind="ExternalInput")
    bv_d = nc.dram_tensor("bv", [1, VC], BF16, kind="ExternalInput")
    ones_d = nc.dram_tensor("onesr", [1, 512], BF16, kind="ExternalInput")
    if has_bias:
        bq_d = nc.dram_tensor("bq", [1, DC], BF16, kind="ExternalInput")
        bk_d = nc.dram_tensor("bk", [1, DC], BF16, kind="ExternalInput")
    out_d = nc.dram_tensor("out", [T, C], BF16, kind="ExternalOutput")

    SCALE = 1.0 / float(np.sqrt(D))
    xT_r = xT_d.rearrange("(ko p) t -> p ko t", p=128)
    cT_r = cT_d.rearrange("(ko p) t -> p ko t", p=128)

    with TileContext(nc) as tc:
        with (
            tc.tile_pool(name="const", bufs=1) as constp,
            tc.tile_pool(name="w", bufs=1) as wpool,
            tc.tile_pool(name="act", bufs=6) as actp,
            tc.tile_pool(name="qkv", bufs=1) as qkvp,
            tc.tile_pool(name="exp", bufs=1) as expp,
            tc.tile_pool(name="y", bufs=1) as yp,
            tc.tile_pool(name="nrm", bufs=6) as nrmp,
            tc.tile_pool(name="ob", bufs=8) as obp,
            tc.tile_pool(name="psQK", bufs=2, space="PSUM") as psQK,
            tc.tile_pool(name="psAV", bufs=1, space="PSUM") as psAV,
            tc.tile_pool(name="psP", bufs=2, space="PSUM") as psP,
        ):
            # ---- first-needed weights ----
            wk = wpool.tile([128, KO, DC], BF16, tag="wk")
            wv = wpool.tile([128, KO, VC], BF16, tag="wv")
            wk_r = wk_d.rearrange("(ko p) d -> p ko d", p=128)
            nc.sync.dma_start(wk[:, 0:2, 0:128], wk_r[:, 0:2, 0:128])
            # loaded later, between projection passes (hides under compute)
            msk = constp.tile([128, 128], F32R, tag="msk")
            idn = constp.tile([128, 128], BF16, tag="idn")
            wq = wpool.tile([128, KO, DC], BF16, tag="wq")
            wp = wpool.tile([128, 2, C], BF16, tag="wp")
            ones = constp.tile([1, 512], BF16, tag="ones")
            bv = constp.tile([1, VC], BF16, tag="bv")
            if has_bias:
                bq = constp.tile([1, DC], BF16, tag="bq")
                bk = constp.tile([1, DC], BF16, tag="bk")
                nc.sync.dma_start(bk[:], bk_d[:])

            # ---- persistent activations ----
            qT = qkvp.tile([128, 2, T], BF16, tag="qT")          # [dc, m, t]
            kT = qkvp.tile([128, 2, T], BF16, tag="kT")
            vA = qkvp.tile([128, NT, HC * 65], BF16, tag="vA")   # v_aug
            yT = yp.tile([128, 2, T], BF16, tag="yT")
            # manual PSUM sub-slot rings in two banks: six [128,128] f32
            # AV-accumulator slots + four [128,128] bf16 transpose slots
            # (bank-granular pools could not provide this depth)
            avb1 = psAV.tile([128, 512], F32, tag="avb1")
            avb2 = psAV.tile([128, 512], F32, tag="avb2")
            av_slots = [avb1[:, 128 * k: 128 * (k + 1)] for k in range(4)]
            av_slots += [avb2[:, 0:128], avb2[:, 128:256]]
            ptr_slots = [avb2[:, 256 + 64 * k: 320 + 64 * k].bitcast(BF16)
                         for k in range(4)]
            ring = {'av': 0, 'ptr': 0}
            # per-phase ping-pong expT buffers (heads alternate)
            expT = []
            for p_ in range(3):
                pair = []
                for ab in ("ab" if p_ == 0 else "abc"):
                    t_ = expp.tile([128, PH[p_]["COLS"]], BF16,
                                   name=f"expT{p_}{ab}", tag=f"expT{p_}{ab}")
                    pair.append(t_)
                expT.append(pair)
            if not has_bias:
                # softmax-denominator ones columns, set once; v_chunk
                # copies then skip them (saves 16 PE bias matmuls)
                for h in range(HC):
                    nc.vector.memset(
                        vA[:, :, 64 + 65 * h: 65 + 65 * h], 1.0)

            def load_chunk(src_r, tc4, after_q0=None, eighths=False):
                # one 512-wide t-chunk of the [c, t] source, as 4 quarter-ko
                # tiles (faster slot turnover at chunk boundaries); the very
                # first chunk loads in ko-eighths so the k-loop starts as
                # soon as one subtile lands
                cs = slice(512 * tc4, 512 * (tc4 + 1))
                tiles = []
                for q in range(2):
                    a = actp.tile([128, 4, 512], BF16, tag="a_in")
                    nc.sync.dma_start(a[:], src_r[:, 4 * q: 4 * q + 4, cs])
                    tiles.append(a)
                    if q == 0 and after_q0 is not None:
                        after_q0()

                def kslice(k, sub=None):
                    t_ = tiles[k // 4]
                    if sub is None:
                        return t_[:, k % 4, :]
                    return t_[:, k % 4, 128 * sub: 128 * (sub + 1)]
                return kslice

            def qk_chunk(kslice, w_t, b_t, dst, tc4, ms=(0, 1)):
                # dst[:, m, chunk] [128 dc, 512 t] = (w^T x)^T + bias
                for m in ms:
                    ps = psP.tile([128, 512], F32, tag="psP")
                    for k in range(KO):
                        nc.tensor.matmul(
                            ps[:], w_t[:, k, 128 * m: 128 * (m + 1)],
                            kslice(k), start=(k == 0),
                            stop=(not has_bias and k == KO - 1))
                    if has_bias:
                        nc.tensor.matmul(
                            ps[:], b_t[:, 128 * m: 128 * (m + 1)],
                            ones[:], start=False, stop=True)
                    nc.vector.tensor_copy(
                        dst[:, m, 512 * tc4: 512 * (tc4 + 1)], ps[:])

            def v_chunk(kslice, tc4, subs=(0, 1, 2, 3)):
                for sub in subs:
                    j = 4 * tc4 + sub
                    ps = psP.tile([128, 512], F32, tag="psP")
                    for k in range(KO):
                        nc.tensor.matmul(
                            ps[:, :VC], kslice(k, sub),
                            wv[:, k, :], start=(k == 0),
                            stop=(not has_bias and k == KO - 1))
                    if has_bias:
                        # bv_aug also carries the v_aug ones column
                        nc.tensor.matmul(
                            ps[:, :VC], ones[:, :128], bv[:],
                            start=False, stop=True)
                        nc.vector.tensor_copy(vA[:, j, :], ps[:, :VC])
                    else:
                        # strided copy skips the ones columns
                        dst = vA[:, j, :]
                        dst = type(dst)(dst.tensor, dst.offset,
                                        [dst.ap[0], [65, 4], [1, 64]])
                        src = ps[:, 0:VC]
                        src = type(src)(src.tensor, src.offset,
                                        [src.ap[0], [65, 4], [1, 64]])
                        nc.vector.tensor_copy(dst, src)

            def qk_group(h, ph, group):
                m, lo = h // 2, 64 * (h % 2)
                kT_h = kT[lo:lo + 64, m, :]
                qT_h = qT[lo:lo + 64, m, :]
                P = PH[ph]
                TLO, OFF = P["TLO"], P["OFF"]
                eT = expT[ph][h % len(expT[ph])]
                ps = psQK.tile([128, 1024], F32, tag="psQK")
                # chunk 0 right-aligned against offset 512, chunk 1 at 512:
                # both stay inside their PSUM bank and the pair is
                # contiguous, so ANY two chunks share one exp instruction
                base = 512 - group[0][2] if len(group) == 2 else 0
                off = base
                for (j, c0, w, eo) in group:
                    s0 = max(128 * j, TLO)
                    nc.tensor.matmul(
                        ps[:, off: off + w],
                        kT_h[:, 128 * j: 128 * (j + 1)],
                        qT_h[:, s0 + c0: s0 + c0 + w],
                        start=True, stop=True)
                    off += w
                total = sum(g[2] for g in group)
                eo0 = group[0][3]
                nc.scalar.activation(
                    eT[:, eo0: eo0 + total], ps[:, base: base + total],
                    EXP, scale=SCALE)
                for (j, c0, w, eo) in group:
                    if c0 == 0 and max(128 * j, TLO) == 128 * j:
                        # diagonal 128x128 at span start: keep s <= t
                        meng = nc.vector if ph == 2 else nc.gpsimd
                        meng.tensor_mul(
                            eT[:, OFF[j]: OFF[j] + 128],
                            eT[:, OFF[j]: OFF[j] + 128], msk[:])

            def qk_units(h, ph):
                return [(lambda g=g: qk_group(h, ph, g))
                        for g in PH[ph]["groups"]]

            def av_part(h, ph, fills=None, alt_tail=False, nxt=None,
                        tail_tiles=None):
                # Flipped AV: per 128-row t-tile i, accumulate
                # out[t, 65] += expblock(j,i)^T @ v_aug(j) over s-tiles
                # j<=i. The exp block is the PE-stationary operand, so only
                # 65 v columns stream per (i,j) pair (vs 512 t-cols in the
                # direct orientation) — half the AV cycles. Output lands
                # t-on-partitions, so softmax normalization is a
                # per-partition tensor_scalar_mul (no broadcast matmul),
                # then a PE transpose puts yT back in [dims, t] layout for
                # the out-projection. The NEXT head's QK groups and fill
                # work interleave between tiles so ACT/PE stay busy.
                m, lo = h // 2, 64 * (h % 2)
                vA_h_base = 65 * h
                P = PH[ph]
                TLO, OFF = P["TLO"], P["OFF"]
                eT = expT[ph][h % len(expT[ph])]
                nxt = list(nxt) if nxt else []
                pend = None   # (i, pAV, ynorm) awaiting transpose+copy

                def flush(pend):
                    i, pAV, ynorm = pend
                    ptr = ptr_slots[ring['ptr'] % 4]
                    ring['ptr'] += 1
                    nc.tensor.transpose(ptr[:64, :], ynorm[:], idn[:])
                    nc.vector.tensor_copy(
                        yT[lo:lo + 64, m, 128 * i: 128 * (i + 1)],
                        ptr[:64, :])
                    if tail_tiles is not None and i in tail_tiles:
                        out_proj_tile(i, alt=True)

                ntiles = P["THI"] // 128 - TLO // 128
                tleft = ntiles
                for i in range(TLO // 128, P["THI"] // 128):
                    k = (-(-len(nxt) // tleft) + 1) if nxt else 0
                    for _ in range(min(k, len(nxt))):
                        nxt.pop(0)()
                    tleft -= 1
                    pAV = av_slots[ring['av'] % 6]
                    ring['av'] += 1
                    for j in range(i + 1):
                        blk = OFF[j] + 128 * i - max(128 * j, TLO)
                        nc.tensor.matmul(
                            pAV[:, :VC - 195], eT[:, blk: blk + 128],
                            vA[:, j, vA_h_base: vA_h_base + 65],
                            start=(j == 0), stop=(j == i))
                    rcp = nrmp.tile([128, 1], F32, tag="rcp")
                    nc.vector.reciprocal(rcp[:], pAV[:, 64:65])
                    ynorm = nrmp.tile([128, 64], BF16, tag="ynorm")
                    nc.vector.tensor_scalar_mul(ynorm[:], pAV[:, 0:64],
                                                rcp[:])
                    if pend is not None:
                        flush(pend)
                    pend = (i, pAV, ynorm)
                    if fills is not None and i in fills:
                        fills[i]()
                flush(pend)
                while nxt:
                    nxt.pop(0)()
                if fills is not None and 'end' in fills:
                    fills['end']()

            def attn_phase(h, ph, fills=None, alt_tail=False, nxt=None):
                for u in qk_units(h, ph):
                    u()
                av_part(h, ph, fills=fills, alt_tail=alt_tail, nxt=nxt)

            def out_proj_tile(tt, alt=False):
                ts = slice(128 * tt, 128 * (tt + 1))
                if alt:
                    ob2 = obp.tile([128, 1024], BF16, tag="ob2")
                else:
                    ob2 = None
                for co in range(2):
                    ps = psP.tile([128, 512], F32, tag="psP")
                    for m in range(2):
                        nc.tensor.matmul(
                            ps[:], yT[:, m, ts],
                            wp[:, m, 512 * co: 512 * (co + 1)],
                            start=(m == 0), stop=(m == 1))
                    if alt:
                        # tail: both halves into one ob, single output DMA
                        # (one less HWDGE dispatch + sem on the drain path)
                        cs = slice(512 * co, 512 * (co + 1))
                        if (tt + co) % 2:
                            nc.scalar.activation(ob2[:, cs], ps[:], COPY)
                        else:
                            nc.vector.tensor_copy(ob2[:, cs], ps[:])
                    else:
                        ob = obp.tile([128, 512], BF16, tag="ob")
                        nc.vector.tensor_copy(ob[:], ps[:])
                        nc.sync.dma_start(
                            out_d[ts, 512 * co: 512 * (co + 1)], ob[:])
                if alt:
                    nc.sync.dma_start(out_d[ts, :], ob2[:])

            # ---- emission ----
            def proj_pair(tc4, first=False):
                def _rest():
                    nc.sync.dma_start(wk[:, 2:KO, 0:128], wk_r[:, 2:KO, 0:128])
                    nc.sync.dma_start(ones[:], ones_d[:])
                ksc = load_chunk(cT_r, tc4, after_q0=_rest if first else None)
                if first:
                    wv_r = wv_d.rearrange("(ko p) d -> p ko d", p=128)
                    nc.sync.dma_start(wk[:, :, 128:256], wk_r[:, :, 128:256])
                    nc.sync.dma_start(wv[:, 0:4, :], wv_r[:, 0:4, :])
                    nc.sync.dma_start(wv[:, 4:KO, :], wv_r[:, 4:KO, :])
                    nc.sync.dma_start(bv[:], bv_d[:])
                def _wq():
                    nc.sync.dma_start(
                        wq[:], wq_d.rearrange("(ko p) d -> p ko d", p=128))
                ksx = load_chunk(xT_r, tc4, after_q0=_wq if first else None)
                if first:
                    if has_bias:
                        nc.sync.dma_start(bq[:], bq_d[:])
                    nc.sync.dma_start(msk[:], msk_d[:])
                    nc.sync.dma_start(idn[:], idn_d[:])
                qk_chunk(ksc, wk, bk if has_bias else None, kT, tc4)
                v_chunk(ksc, tc4)
                qk_chunk(ksx, wq, bq if has_bias else None, qT, tc4)

            proj_pair(0, first=True)
            proj_pair(1)

            # phase A (t<1024, tiles 0-7): proj chunk 2 fills the exp gaps
            ksc2 = load_chunk(cT_r, 2)
            ksx2 = load_chunk(xT_r, 2)
            bkk = bk if has_bias else None
            bqq = bq if has_bias else None
            st3 = {}

            def _loads3():
                st3['c'] = load_chunk(cT_r, 3)
                st3['x'] = load_chunk(xT_r, 3)

            def _wp_dma():
                nc.sync.dma_start(
                    wp[:], wp_d.rearrange("(m p) c -> p m c", p=128))

            for u in qk_units(0, 0):
                u()
            av_part(0, 0, fills={3: lambda: qk_chunk(ksc2, wk, bkk, kT, 2)},
                    nxt=qk_units(1, 0))
            av_part(1, 0, fills={3: lambda: v_chunk(ksc2, 2)},
                    nxt=qk_units(2, 0))
            av_part(2, 0, fills={3: lambda: qk_chunk(ksx2, wq, bqq, qT, 2),
                                 'end': _loads3},
                    nxt=qk_units(3, 0))
            av_part(3, 0, fills={3: lambda: qk_chunk(st3['c'], wk, bkk, kT, 3),
                                 'end': _wp_dma},
                    nxt=qk_units(0, 1))

            # phase B (t in [1024,1536), tiles 8-11): proj chunk 3 pieces +
            # first out-proj tiles fill the gaps
            av_part(0, 1, fills={9: lambda: v_chunk(st3['c'], 3, subs=(0, 1))},
                    nxt=qk_units(1, 1))
            av_part(1, 1, fills={
                9: lambda: v_chunk(st3['c'], 3, subs=(2, 3)),
                10: lambda: qk_chunk(st3['x'], wq, bqq, qT, 3, ms=(0,)),
            }, nxt=qk_units(2, 1))
            av_part(2, 1, fills={
                9: lambda: qk_chunk(st3['x'], wq, bqq, qT, 3, ms=(1,)),
                10: lambda: out_proj_tile(0),
            }, nxt=qk_units(3, 1))
            av_part(3, 1, fills={9: lambda: [out_proj_tile(t) for t in (1, 2)]},
                    nxt=qk_units(0, 2))

            # phase C (t in [1536,2048), tiles 12-15): remaining tiles fill;
            # the last head emits tiles 12-15 as each t-slice normalizes
            av_part(0, 2, fills={13: lambda: [out_proj_tile(t) for t in (3, 4)],
                                 15: lambda: [out_proj_tile(t) for t in (5,)]},
                    nxt=qk_units(1, 2))
            av_part(1, 2, fills={13: lambda: [out_proj_tile(t) for t in (6, 7)],
                                 15: lambda: [out_proj_tile(t) for t in (8,)]},
                    nxt=qk_units(2, 2))
            av_part(2, 2, fills={13: lambda: [out_proj_tile(t) for t in (9, 10)],
                                 15: lambda: [out_proj_tile(t) for t in (11,)]},
                    nxt=qk_units(3, 2))
            av_part(3, 2, alt_tail=True, tail_tiles={12, 13, 14, 15})

    split_sync_waits(nc)
    return nc


def kernel(x, context, Wq, bq, Wkv, bkv, Wp, bp):
    from concourse.bass_utils import run_bass_kernel_spmd

    x = np.asarray(x, dtype=np.float32)
    context = np.asarray(context, dtype=np.float32)
    Wq = np.asarray(Wq, dtype=np.float32)
    Wkv = np.asarray(Wkv, dtype=np.float32)
    Wp_a = np.asarray(Wp, dtype=np.float32)
    bq_a = np.asarray(bq, dtype=np.float32)
    bkv_a = np.asarray(bkv, dtype=np.float32)
    bp_a = np.asarray(bp, dtype=np.float32)

    has_bias = bool(np.any(bq_a) or np.any(bkv_a))
    key = ('nc', has_bias)
    if key not in _cached:
        _cached[key] = build_program(has_bias)
    nc = _cached[key]

    bf16 = ml_dtypes.bfloat16
    msk = np.triu(np.ones((128, 128), dtype=np.float32))
    xT = [np.ascontiguousarray(x[b].T).astype(bf16) for b in range(B)]
    cT = [np.ascontiguousarray(context[b].T).astype(bf16) for b in range(B)]

    ones512 = np.ones((1, 512), dtype=bf16)
    in_maps = []
    for c in range(8):
        b, hg = c // 4, c % 4
        hs = slice(DC * hg, DC * (hg + 1))
        wv_aug = np.zeros((C, VC), dtype=np.float32)
        bv_aug = np.zeros((1, VC), dtype=np.float32)
        for h in range(HC):
            wv_aug[:, 65 * h: 65 * h + 64] = Wkv[:, C + DC * hg + 64 * h:
                                                 C + DC * hg + 64 * (h + 1)]
            bv_aug[0, 65 * h: 65 * h + 64] = bkv_a[C + DC * hg + 64 * h:
                                                   C + DC * hg + 64 * (h + 1)]
            bv_aug[0, 65 * h + 64] = 1.0
        im = {
            "xT": xT[b], "cT": cT[b],
            "wq": np.ascontiguousarray(Wq[:, hs]).astype(bf16),
            "wk": np.ascontiguousarray(Wkv[:, hs]).astype(bf16),
            "wv": wv_aug.astype(bf16),
            "wp": np.ascontiguousarray(Wp_a[hs, :]).astype(bf16),
            "msk": msk, "idn": np.eye(128, dtype=np.float32).astype(bf16),
            "bv": bv_aug.astype(bf16), "onesr": ones512,
        }
        if has_bias:
            im.update({
                "bq": np.ascontiguousarray(bq_a[hs]).reshape(1, DC).astype(bf16),
                "bk": np.ascontiguousarray(bkv_a[hs]).reshape(1, DC).astype(bf16),
            })
        in_maps.append(im)

    res = run_bass_kernel_spmd(nc, in_maps, list(range(8)))
    out = np.zeros((B, T, C), dtype=np.float32)
    for c in range(8):
        out[c // 4] += np.asarray(res.results[c]["out"], dtype=np.float32)
    out += bp_a[None, None, :]
    return out
